# revision 4
# baseline (speedup 1.0000x reference)
import sys
sys.path.insert(0, '/opt/trn_rl_repo')
import numpy as np

# ActionDetectionModel: TimeDistributed VGG19 (64x64x3) -> 2x bidir peephole LSTM
# -> dense(1024)+relu -> dense(101)+softmax.
# Sharding: one video (16 frames) per core for the VGG; features AllGathered,
# LSTM/dense computed batch-8 redundantly on every core.

F = 16            # frames per core
W1, W2, W3, W4, W5 = 64, 32, 16, 8, 4
P1, P2, P3, P4, P5 = 66, 34, 18, 10, 6
A1, A2, A3, A4, A5 = P1 * P1, P2 * P2, P3 * P3, P4 * P4, P5 * P5
ZC = 1296         # zero-fill chunk (cols)

_CACHE = {}


def _build():
    import concourse.mybir as mybir
    from concourse import bacc, tile

    F32R = mybir.dt.float32r
    F32 = mybir.dt.float32
    AF = mybir.ActivationFunctionType

    nc = bacc.Bacc()
    frames_d = nc.declare_dram_parameter("frames", [F, 3, 64, 64], F32R, isOutput=False)
    v_d = [nc.declare_dram_parameter("v0", [27, 64], F32R, isOutput=False)]
    vgg_io = [(3, 64), (64, 64), (64, 128), (128, 128),
              (128, 256), (256, 256), (256, 256), (256, 256),
              (256, 512), (512, 512), (512, 512), (512, 512),
              (512, 512), (512, 512), (512, 512), (512, 512)]
    for i in range(1, 16):
        cin, cout = vgg_io[i]
        v_d.append(nc.declare_dram_parameter(f"v{i}", [9, cin, cout], F32R, isOutput=False))
    lw = {}
    for d in ("f", "b"):
        lw[f"k1{d}"] = nc.declare_dram_parameter(f"k1{d}", [16, 128, 1024], F32R, isOutput=False)
        lw[f"k2{d}"] = nc.declare_dram_parameter(f"k2{d}", [4, 128, 1024], F32R, isOutput=False)
        for l in (1, 2):
            lw[f"rk{l}{d}"] = nc.declare_dram_parameter(f"rk{l}{d}", [256, 1024], F32R, isOutput=False)
            lw[f"p{l}{d}"] = nc.declare_dram_parameter(f"p{l}{d}", [3, 256, 256], F32R, isOutput=False)
    w1_d = nc.declare_dram_parameter("w1", [512, 1024], F32R, isOutput=False)
    w2_d = nc.declare_dram_parameter("w2p", [1024, 104], F32R, isOutput=False)
    out_d = nc.declare_dram_parameter("out", [8, 101], F32, isOutput=True)

    def iv(t, y, x):  # [p, (y x)] view
        return t[:].rearrange("p (y x) -> p y x", y=y, x=x)

    def cv(t, c, f, y, x):  # [p, (c f y x)] view
        return t[:].rearrange("p (c f y x) -> p c f y x", c=c, f=f, y=y, x=x)

    with tile.TileContext(nc) as tc:
        with tc.tile_pool(name="glob", bufs=1) as glob:
            zsrc = glob.tile([128, ZC], F32)
            nc.any.memset(zsrc[:], 0.0)

            def zero(t, cols, parts=128):
                for off in range(0, cols, ZC):
                    n = min(ZC, cols - off)
                    nc.any.tensor_copy(t[0:parts, off:off + n], zsrc[0:parts, 0:n])

            b3in = glob.tile([128, F * A3], F32R)
            b4in = glob.tile([128, 2 * F * A4], F32R)
            b5in = glob.tile([128, 4 * F * A5], F32R)
            feats = glob.tile([128, 4 * F * 4], F32R)
            featsT = glob.tile([128, 16 * 128], F32R)
            zx = {(l, d): glob.tile([128, F * 64], F32R, name=f"zx{l}{d}")
                  for l in (1, 2) for d in ("f", "b")}
            hs = {(l, d): glob.tile([128, F * 16], F32R, name=f"hs{l}{d}")
                  for l in (1, 2) for d in ("f", "b")}
            x1T = glob.tile([128, 64], F32R)
            z0t = glob.tile([128, 16], F32R)
            zero(b3in, F * A3)
            zero(b4in, 2 * F * A4)
            zero(b5in, 4 * F * A5)
            zero(z0t, 16)

            # ---------------- streamed conv (blocks 3-5) ----------------
            def conv_stream(wpool, pspool, wparam, src, dst, cin, cout, Wsp, G):
                Pp = Wsp + 2
                kc, mc = cin // 128 if cin >= 128 else 1, cout // 128
                srcv = cv(src, kc, F, Pp, Pp)
                dstv = cv(dst, mc, F, Pp, Pp)
                A = Wsp * Wsp
                fpt = min(F, max(1, 512 // A))
                N = fpt * A
                tiles = [(f0, fpt) for f0 in range(0, F, fpt)]
                for m in range(mc):
                    for g0 in range(0, len(tiles), G):
                        grp = tiles[g0:g0 + G]
                        pss = [pspool.tile([128, N], F32, tag="cps", name="cps")
                               for _ in grp]
                        for ki in range(kc):
                            for tap in range(9):
                                ky, kx = tap // 3, tap % 3
                                wt = wpool.tile([128, 128], F32R, tag="wst", name="wst")
                                nc.sync.dma_start(
                                    wt[:], wparam[tap, ki * 128:(ki + 1) * 128,
                                                  m * 128:(m + 1) * 128])
                                first = (ki == 0 and tap == 0)
                                last = (ki == kc - 1 and tap == 8)
                                for ti, (f0, fp) in enumerate(grp):
                                    rhs = srcv[:, ki, f0:f0 + fp,
                                               ky:ky + Wsp, kx:kx + Wsp]
                                    nc.tensor.matmul(pss[ti][:], wt[:], rhs,
                                                     start=first, stop=last)
                        for ti, (f0, fp) in enumerate(grp):
                            pin = pss[ti][:].rearrange("m (f y x) -> m f y x",
                                                       f=fp, y=Wsp, x=Wsp)
                            out1 = dstv[:, m, f0:f0 + fp, 1:1 + Wsp, 1:1 + Wsp]
                            nc.any.tensor_relu(out1, pin)

            # ---------------- maxpool 2x2 ----------------
            def pool2(scr, srcv, dstv, nch, nf, Wi, padded_out=True, parts=128):
                Wo = Wi // 2
                for ch in range(nch):
                    tmp = scr.tile([parts, nf * Wi * Wo], F32R, tag="ptmp", name="ptmp")
                    tv = tmp[:].rearrange("p (f y x) -> p f y x", f=nf, y=Wi, x=Wo)
                    nc.any.tensor_max(tv, srcv[:, ch, :, 1:1 + Wi, 1:1 + Wi:2],
                                      srcv[:, ch, :, 1:1 + Wi, 2:2 + Wi:2])
                    if padded_out:
                        o = dstv[:, ch, :, 1:1 + Wo, 1:1 + Wo]
                    else:
                        o = dstv[:, ch, :, :, :]
                    nc.any.tensor_max(o, tv[:, :, 0:Wi:2, :], tv[:, :, 1:Wi:2, :])

            # ============ frame loop: blocks 1-2 ============
            with tc.tile_pool(name="pf", bufs=1) as pf, \
                 tc.tile_pool(name="pfp", bufs=2, space="PSUM") as pfp, \
                 tc.tile_pool(name="pfs", bufs=2) as pfs:
                tapA = pf.tile([27, 4096], F32R)
                tapB = pf.tile([27, 4096], F32R)
                b1a = pf.tile([64, A1], F32R)
                b1b = pf.tile([64, A1], F32R)
                b2in = pf.tile([64, A2], F32R)
                b2a = pf.tile([128, A2], F32R)
                b2b = pf.tile([128, A2], F32R)
                for t, cc, pp in ((tapA, 4096, 27), (tapB, 4096, 27), (b1a, A1, 64),
                                  (b1b, A1, 64), (b2in, A2, 64), (b2a, A2, 128),
                                  (b2b, A2, 128)):
                    zero(t, cc, pp)
                w11 = pf.tile([27, 64], F32R)
                nc.sync.dma_start(w11[:], v_d[0][:])
                w12 = [pf.tile([64, 64], F32R, name=f"w12_{t}") for t in range(9)]
                w21 = [pf.tile([64, 128], F32R, name=f"w21_{t}") for t in range(9)]
                w22 = [pf.tile([128, 128], F32R, name=f"w22_{t}") for t in range(9)]
                for t in range(9):
                    nc.sync.dma_start(w12[t][:], v_d[1][t])
                    nc.sync.dma_start(w21[t][:], v_d[2][t])
                    nc.sync.dma_start(w22[t][:], v_d[3][t])

                def conv_res(wt, srcv, dstv, cout, Wsp, parts_in):
                    rpt = 512 // Wsp
                    for y0 in range(0, Wsp, rpt):
                        ps = pfp.tile([cout, 512], F32, tag=f"ps{cout}",
                                      name=f"ps{cout}")
                        for tap in range(9):
                            ky, kx = tap // 3, tap % 3
                            rhs = srcv[0:parts_in, y0 + ky:y0 + ky + rpt,
                                       kx:kx + Wsp]
                            nc.tensor.matmul(ps[:], wt[tap][:], rhs,
                                             start=(tap == 0), stop=(tap == 8))
                        pin = ps[:].rearrange("m (y x) -> m y x", y=rpt, x=Wsp)
                        nc.any.tensor_relu(dstv[:, y0 + 1:y0 + 1 + rpt, 1:1 + Wsp],
                                           pin)

                b3inv = cv(b3in, 1, F, P3, P3)
                for f in range(F):
                    tap = tapA if f % 2 == 0 else tapB
                    frv = frames_d[f]
                    tv4 = tap[:].rearrange("p (y x) -> p y x", y=64, x=64)
                    for ti in range(9):
                        dy, dx = ti // 3 - 1, ti % 3 - 1
                        oy0, oy1 = max(0, -dy), 64 - max(0, dy)
                        ox0, ox1 = max(0, -dx), 64 - max(0, dx)
                        nc.sync.dma_start(
                            tv4[3 * ti:3 * ti + 3, oy0:oy1, ox0:ox1].opt(),
                            frv[:, oy0 + dy:oy1 + dy, ox0 + dx:ox1 + dx].opt())
                    b1av = iv(b1a, P1, P1)
                    for yi in range(8):
                        ps = pfp.tile([64, 512], F32, tag="ps64c1", name="ps64c1")
                        nc.tensor.matmul(ps[:], w11[:],
                                         tap[:, yi * 512:(yi + 1) * 512],
                                         start=True, stop=True)
                        pin = ps[:].rearrange("m (y x) -> m y x", y=8, x=64)
                        nc.any.tensor_relu(
                            b1av[:, yi * 8 + 1:yi * 8 + 9, 1:65], pin)
                    conv_res(w12, b1av, iv(b1b, P1, P1), 64, W1, 64)
                    pool2(pfs, cv(b1b, 1, 1, P1, P1), cv(b2in, 1, 1, P2, P2),
                          1, 1, W1, parts=64)
                    conv_res(w21, iv(b2in, P2, P2), iv(b2a, P2, P2), 128, W2, 64)
                    conv_res(w22, iv(b2a, P2, P2), iv(b2b, P2, P2), 128, W2, 128)
                    pool2(pfs, cv(b2b, 1, 1, P2, P2),
                          cv(b3in, 1, F, P3, P3)[:, :, f:f + 1], 1, 1, W2)

            # ============ block 3 ============
            with tc.tile_pool(name="p3", bufs=1) as p3, \
                 tc.tile_pool(name="ws3", bufs=12) as ws3, \
                 tc.tile_pool(name="cp3", bufs=6, space="PSUM") as cp3, \
                 tc.tile_pool(name="sc3", bufs=2) as sc3:
                b3a = p3.tile([128, 2 * F * A3], F32R)
                b3b = p3.tile([128, 2 * F * A3], F32R)
                zero(b3a, 2 * F * A3)
                zero(b3b, 2 * F * A3)
                conv_stream(ws3, cp3, v_d[4], b3in, b3a, 128, 256, W3, 4)
                conv_stream(ws3, cp3, v_d[5], b3a, b3b, 256, 256, W3, 4)
                conv_stream(ws3, cp3, v_d[6], b3b, b3a, 256, 256, W3, 4)
                conv_stream(ws3, cp3, v_d[7], b3a, b3b, 256, 256, W3, 4)
                pool2(sc3, cv(b3b, 2, F, P3, P3), cv(b4in, 2, F, P4, P4), 2, F, W3)

            # ============ block 4 ============
            with tc.tile_pool(name="p4", bufs=1) as p4, \
                 tc.tile_pool(name="ws4", bufs=12) as ws4, \
                 tc.tile_pool(name="cp4", bufs=6, space="PSUM") as cp4, \
                 tc.tile_pool(name="sc4", bufs=2) as sc4:
                b4a = p4.tile([128, 4 * F * A4], F32R)
                b4b = p4.tile([128, 4 * F * A4], F32R)
                zero(b4a, 4 * F * A4)
                zero(b4b, 4 * F * A4)
                conv_stream(ws4, cp4, v_d[8], b4in, b4a, 256, 512, W4, 2)
                conv_stream(ws4, cp4, v_d[9], b4a, b4b, 512, 512, W4, 2)
                conv_stream(ws4, cp4, v_d[10], b4b, b4a, 512, 512, W4, 2)
                conv_stream(ws4, cp4, v_d[11], b4a, b4b, 512, 512, W4, 2)
                pool2(sc4, cv(b4b, 4, F, P4, P4), cv(b5in, 4, F, P5, P5), 4, F, W4)

            # ============ block 5 ============
            with tc.tile_pool(name="p5", bufs=1) as p5, \
                 tc.tile_pool(name="ws5", bufs=12) as ws5, \
                 tc.tile_pool(name="cp5", bufs=6, space="PSUM") as cp5, \
                 tc.tile_pool(name="sc5", bufs=2) as sc5:
                b5a = p5.tile([128, 4 * F * A5], F32R)
                b5b = p5.tile([128, 4 * F * A5], F32R)
                zero(b5a, 4 * F * A5)
                zero(b5b, 4 * F * A5)
                conv_stream(ws5, cp5, v_d[12], b5in, b5a, 512, 512, W5, 1)
                conv_stream(ws5, cp5, v_d[13], b5a, b5b, 512, 512, W5, 1)
                conv_stream(ws5, cp5, v_d[14], b5b, b5a, 512, 512, W5, 1)
                conv_stream(ws5, cp5, v_d[15], b5a, b5b, 512, 512, W5, 1)
                featv = feats[:].rearrange("p (c y x f) -> p c f y x",
                                           c=4, y=2, x=2, f=F)
                pool2(sc5, cv(b5b, 4, F, P5, P5), featv, 4, F, W5,
                      padded_out=False)

            # ============ AllGather features ============
            with tc.tile_pool(name="dr", bufs=1, space="DRAM") as dr:
                feats_l = dr.tile([128, 256], F32R)
                feats_all = dr.tile([8, 128, 256], F32R, addr_space="Shared")
                nc.sync.dma_start(feats_l[:], feats[:])
                nc.gpsimd.collective_compute(
                    "AllGather", mybir.AluOpType.bypass,
                    replica_groups=[list(range(8))],
                    ins=[feats_l[:].opt()], outs=[feats_all[:].opt()])
                agv = feats_all[:].rearrange("v p (c yx f) -> p c yx v f",
                                             c=4, yx=4, f=F)
                ftv = featsT[:].rearrange("p (j b t) -> p j b t", j=16, b=8, t=16)
                for j in range(16):
                    nc.sync.dma_start(ftv[:, j], agv[:, j % 4, j // 4])

            # ============ LSTM + dense ============
            with tc.tile_pool(name="pl", bufs=1) as pl, \
                 tc.tile_pool(name="wl", bufs=8) as wl, \
                 tc.tile_pool(name="lp", bufs=4, space="PSUM") as lp, \
                 tc.tile_pool(name="ls", bufs=3) as ls:
                rkt = {(l, d): [pl.tile([128, 1024], F32R, name=f"rk{l}{d}{k}")
                                for k in range(2)]
                       for l in (1, 2) for d in ("f", "b")}
                ppt = {(l, d): [[pl.tile([128, 256], F32R, name=f"pp{l}{d}{g}{k}")
                                 for k in range(2)] for g in range(3)]
                       for l in (1, 2) for d in ("f", "b")}
                for l in (1, 2):
                    for d in ("f", "b"):
                        for k in range(2):
                            nc.sync.dma_start(rkt[(l, d)][k][:],
                                              lw[f"rk{l}{d}"][k * 128:(k + 1) * 128, :])
                            for g in range(3):
                                nc.sync.dma_start(
                                    ppt[(l, d)][g][k][:],
                                    lw[f"p{l}{d}"][g, k * 128:(k + 1) * 128, :])

                def inproj(l, d, nk, rhs_fn, bt):
                    zv = zx[(l, d)][:].rearrange("p (t m b) -> p m b t",
                                                 t=F, m=8, b=8)
                    kp = lw[f"k{l}{d}"]
                    for m in range(8):
                        psz = lp.tile([128, 128], F32, tag="lps", name="lpsz")
                        for j in range(nk):
                            kt = wl.tile([128, 128], F32R, tag="kt", name="kt")
                            nc.sync.dma_start(kt[:], kp[j, :, m * 128:(m + 1) * 128])
                            nc.tensor.matmul(psz[:], kt[:], rhs_fn(j),
                                             start=(j == 0), stop=(j == nk - 1))
                        if bt:
                            pv = psz[:].rearrange("p (b t) -> p b t", b=8, t=F)
                        else:
                            pv = psz[:].rearrange("p (t b) -> p b t", t=F, b=8)
                        nc.any.tensor_copy(zv[:, m], pv)

                def scan(l, d):
                    hsb = hs[(l, d)]
                    rk = rkt[(l, d)]
                    pp = ppt[(l, d)]
                    zxb = zx[(l, d)]
                    cprev = None
                    for i in range(F):
                        t = i if d == "f" else F - 1 - i
                        tp = t - 1 if d == "f" else t + 1
                        psz = lp.tile([128, 64], F32, tag="lps", name="lpss")
                        hrhs = (lambda k: z0t[:, k * 8:(k + 1) * 8]) if i == 0 else \
                               (lambda k: hsb[:, tp * 16 + k * 8:tp * 16 + (k + 1) * 8])
                        crhs = (lambda k: z0t[:, k * 8:(k + 1) * 8]) if i == 0 else \
                               (lambda k, c=cprev: c[:, k * 8:(k + 1) * 8])
                        for m in range(8):
                            for k in range(2):
                                st = (k == 0)
                                sp = (k == 1 and m < 2)
                                nc.tensor.matmul(psz[:, m * 8:(m + 1) * 8],
                                                 rk[k][:, m * 128:(m + 1) * 128],
                                                 hrhs(k), start=st, stop=sp)
                            if m >= 2:
                                g, m2 = (m - 2) // 2, (m - 2) % 2
                                for k in range(2):
                                    nc.tensor.matmul(
                                        psz[:, m * 8:(m + 1) * 8],
                                        pp[g][k][:, m2 * 128:(m2 + 1) * 128],
                                        crhs(k), start=False, stop=(k == 1))
                        zsb = ls.tile([128, 64], F32, tag="zsb", name="zsb")
                        nc.vector.tensor_add(zsb[:], psz[:],
                                             zxb[:, t * 64:(t + 1) * 64])
                        sig = ls.tile([128, 48], F32, tag="sig", name="sig")
                        nc.scalar.activation(sig[:], zsb[:, 16:64], AF.Sigmoid)
                        gt = ls.tile([128, 16], F32, tag="gt", name="gt")
                        nc.scalar.activation(gt[:], zsb[:, 0:16], AF.Tanh)
                        t1 = ls.tile([128, 16], F32, tag="t1", name="t1")
                        c_in = z0t if i == 0 else cprev
                        nc.vector.tensor_mul(t1[:], sig[:, 16:32], c_in[:, 0:16])
                        t2 = ls.tile([128, 16], F32, tag="t2", name="t2")
                        nc.vector.tensor_mul(t2[:], sig[:, 0:16], gt[:])
                        cnew = ls.tile([128, 16], F32R, tag=f"c{l}{d}",
                                       name=f"c{l}{d}")
                        nc.vector.tensor_add(cnew[:], t1[:], t2[:])
                        tc_ = ls.tile([128, 16], F32, tag="tc", name="tc_")
                        nc.scalar.activation(tc_[:], cnew[:], AF.Tanh)
                        nc.vector.tensor_mul(hsb[:, t * 16:(t + 1) * 16],
                                             sig[:, 32:48], tc_[:])
                        cprev = cnew

                for d in ("f", "b"):
                    inproj(1, d, 16, lambda j: featsT[:, j * 128:(j + 1) * 128], True)
                for d in ("f", "b"):
                    scan(1, d)
                h1fv = hs[(1, "f")][:].rearrange("p (t c b) -> p t c b",
                                                 t=F, c=2, b=8)
                h1bv = hs[(1, "b")][:].rearrange("p (t c b) -> p t c b",
                                                 t=F, c=2, b=8)
                for d in ("f", "b"):
                    inproj(2, d, 4, lambda j: (h1fv if j < 2 else h1bv)[:, :, j % 2, :], False)
                for d in ("f", "b"):
                    scan(2, d)

                # dense1 -> relu -> x1T
                w1t = [pl.tile([128, 1024], F32R, name=f"w1t{k}") for k in range(4)]
                for k in range(4):
                    nc.sync.dma_start(w1t[k][:], w1_d[k * 128:(k + 1) * 128, :])
                xr = [hs[(2, "f")][:, 15 * 16 + 0 * 8:15 * 16 + 8],
                      hs[(2, "f")][:, 15 * 16 + 8:15 * 16 + 16],
                      hs[(2, "b")][:, 0:8], hs[(2, "b")][:, 8:16]]
                for m in range(8):
                    psd = lp.tile([128, 8], F32, tag="lps", name="lpsd")
                    for k in range(4):
                        nc.tensor.matmul(psd[:], w1t[k][:, m * 128:(m + 1) * 128],
                                         xr[k], start=(k == 0), stop=(k == 3))
                    nc.any.tensor_relu(x1T[:, m * 8:(m + 1) * 8], psd[:])
                # dense2 + softmax
                w2t = [pl.tile([128, 104], F32R, name=f"w2t{k}") for k in range(8)]
                for k in range(8):
                    nc.sync.dma_start(w2t[k][:], w2_d[k * 128:(k + 1) * 128, :])
                ps2 = lp.tile([8, 104], F32, tag="lps", name="lps2")
                for k in range(8):
                    nc.tensor.matmul(ps2[:], x1T[:, k * 8:(k + 1) * 8], w2t[k][:],
                                     start=(k == 0), stop=(k == 7))
                rmax = ls.tile([8, 1], F32, tag="sm", name="rmax")
                nc.vector.tensor_reduce(rmax[:], ps2[:, 0:101],
                                        axis=mybir.AxisListType.X,
                                        op=mybir.AluOpType.max)
                nmax = ls.tile([8, 1], F32, tag="sm2", name="nmax")
                nc.scalar.mul(nmax[:], rmax[:], -1.0)
                ex = ls.tile([8, 101], F32, tag="ex", name="ex")
                nc.scalar.activation(ex[:], ps2[:, 0:101], AF.Exp, bias=nmax[:])
                ssum = ls.tile([8, 1], F32, tag="sm3", name="ssum")
                nc.vector.tensor_reduce(ssum[:], ex[:], axis=mybir.AxisListType.X,
                                        op=mybir.AluOpType.add)
                rinv = ls.tile([8, 1], F32, tag="sm4", name="rinv")
                nc.vector.reciprocal(rinv[:], ssum[:])
                outsb = ls.tile([8, 101], F32, tag="outsb", name="outsb")
                nc.vector.tensor_scalar_mul(outsb[:], ex[:], rinv[:])
                nc.sync.dma_start(out_d[:], outsb[:])

    nc.finalize()
    return nc


def _prep_weights(vgg, l1f, l1b, l2f, l2b, w1, w2):
    wm = {}
    wm["v0"] = np.ascontiguousarray(vgg[0]['w'].reshape(27, 64), np.float32)
    cins = [None, (64, 64), (64, 128), (128, 128), (128, 256), (256, 256),
            (256, 256), (256, 256), (256, 512), (512, 512), (512, 512),
            (512, 512), (512, 512), (512, 512), (512, 512), (512, 512)]
    for i in range(1, 16):
        cin, cout = cins[i]
        wm[f"v{i}"] = np.ascontiguousarray(
            np.asarray(vgg[i]['w'], np.float32).reshape(9, cin, cout))
    for d, p in (("f", l1f), ("b", l1b)):
        k = np.asarray(p['kernel'], np.float32)          # [2048, 1024] (y,x,c) order
        wm[f"k1{d}"] = np.ascontiguousarray(
            k.reshape(4, 4, 128, 1024).reshape(16, 128, 1024))
        wm[f"rk1{d}"] = np.ascontiguousarray(np.asarray(p['rkernel'], np.float32))
        wm[f"p1{d}"] = np.ascontiguousarray(
            np.stack([np.asarray(p['Wi'], np.float32),
                      np.asarray(p['Wf'], np.float32),
                      np.asarray(p['Wo'], np.float32)]))
    for d, p in (("f", l2f), ("b", l2b)):
        k = np.asarray(p['kernel'], np.float32)          # [512, 1024]
        wm[f"k2{d}"] = np.ascontiguousarray(k.reshape(4, 128, 1024))
        wm[f"rk2{d}"] = np.ascontiguousarray(np.asarray(p['rkernel'], np.float32))
        wm[f"p2{d}"] = np.ascontiguousarray(
            np.stack([np.asarray(p['Wi'], np.float32),
                      np.asarray(p['Wf'], np.float32),
                      np.asarray(p['Wo'], np.float32)]))
    wm["w1"] = np.ascontiguousarray(np.asarray(w1, np.float32))
    w2p = np.zeros((1024, 104), np.float32)
    w2p[:, 0:101] = np.asarray(w2, np.float32)
    wm["w2p"] = w2p
    return wm


def kernel(inputs, vgg, lstm1_fwd, lstm1_bwd, lstm2_fwd, lstm2_bwd,
           w1, b1, w2, b2):
    from concourse.bass_utils import run_bass_kernel_spmd
    if "nc" not in _CACHE:
        _CACHE["nc"] = _build()
    nc = _CACHE["nc"]
    wm = _prep_weights(vgg, lstm1_fwd, lstm1_bwd, lstm2_fwd, lstm2_bwd, w1, w2)
    frames = np.ascontiguousarray(
        np.asarray(inputs, np.float32).transpose(0, 1, 4, 2, 3))  # [8,16,3,64,64]
    in_maps = [{"frames": frames[v], **wm} for v in range(8)]
    res = run_bass_kernel_spmd(nc, in_maps, list(range(8)))
    return np.asarray(res.results[0]["out"], np.float32)
